# revision 35
# baseline (speedup 1.0000x reference)
"""Trainium2 Bass kernel for nn_Drifting_74423193305271 (cosine-similarity loss).

Reference computes, per batch b:
    x = fix_outputs * region_mask          (mask over feature dim)
    G = x @ x.T  (S x S gram), sim = G / (n n^T),  n_t = max(||x_t||, eps)
    loss terms = sum over strict upper triangle of sim, all batches
    out = -log(1 - 0.5*(avg+1)) * 0.1

Key identity: with y_t = x_t / n_t,
    sum_{t<u} sim_tu = 0.5 * (||sum_t y_t||^2 - sum_t ||y_t||^2)
so the O(S^2 D) gram matrix is never needed — one masked-norm pass over the
data plus a weighted column sum (a [1,S] @ [S,D] matmul) suffices.

Device work per core (4 batches of [512, 1024] f32), engine-balanced:
    xm      = bf16(x * mask)                (DVE tensor_mul; mask replicated
                                             once to [128, 4*1024] bf16 SBUF
                                             via K=1 PE matmuls)
    n2[t]   = sum_d xm[t,d]^2               (ACT Square activation, accum_out)
    inv[t]  = 1 / max(sqrt(n2), eps)        (bf16 for the PE)
    s[d]    = sum_t inv[t] * xm[t,d]        (PE bf16 matmul, f32 PSUM accum)
    tr[t]   = n2[t] * round_bf16(inv[t])^2  (diagonal term; uses the SAME
                                             rounded inv the PE consumes so the
                                             diagonal inside ||s||^2 cancels)
Host combines: total = 0.5 * (sum mask*(s^2) - sum tr), then the log penalty.
bf16(x)*m == bf16(x*m) exactly since mask is 0/1.

NB: vector.tensor_tensor_reduce wedges the device (NRT INTERNAL error) on this
stack — avoid it; the tensor_mul + activation(accum_out) split above is the
working equivalent. bf16 matmul matters: fp32 PE streams at ~4 cycles/column.
"""

import sys

import numpy as np

if "/opt/trn_rl_repo" not in sys.path:
    sys.path.insert(0, "/opt/trn_rl_repo")

B, S, D = 32, 512, 1024
N_CORES = 8
B_PER = B // N_CORES  # 4 batches per core
P = 128
T_TILES = S // P  # 4 row tiles of 128 timesteps per batch
N_COLS = B_PER * T_TILES  # 16 stat columns per core
EPS = 1e-8
BETA = 0.1
H = 512  # matmul free-dim half (one PSUM bank)

_compiled_nc = None


def _build(reps: int = 1, loop_n: int = 0):
    """loop_n > 0 wraps the body in a device-side For_i loop (benchmarking
    only — one dispatch then executes the kernel loop_n * reps times)."""
    from contextlib import ExitStack, nullcontext

    import concourse.bass as bass
    import concourse.tile as tile
    from concourse import bacc, mybir

    fp32 = mybir.dt.float32
    bf16 = mybir.dt.bfloat16

    nc = bacc.Bacc(
        "TRN2",
        target_bir_lowering=False,
        debug=False,
        num_devices=N_CORES,
    )

    x_d = nc.dram_tensor("x", [B_PER * S, D], fp32, kind="ExternalInput")
    m_d = nc.dram_tensor("mask", [1, B_PER * D], bf16, kind="ExternalInput")
    s_d = nc.dram_tensor("out_s", [1, B_PER * D], fp32, kind="ExternalOutput")
    tr_d = nc.dram_tensor("out_tr", [P, N_COLS], fp32, kind="ExternalOutput")

    with tile.TileContext(nc) as tc, ExitStack() as ctx:
        x_pool = ctx.enter_context(tc.tile_pool(name="x", bufs=10))
        xm_pool = ctx.enter_context(tc.tile_pool(name="xm", bufs=4 * T_TILES))
        sq_pool = ctx.enter_context(tc.tile_pool(name="sq", bufs=3))
        const_pool = ctx.enter_context(tc.tile_pool(name="const", bufs=1))
        stat_pool = ctx.enter_context(tc.tile_pool(name="stat", bufs=3))
        ssb_pool = ctx.enter_context(tc.tile_pool(name="ssb", bufs=2))
        spsum_pool = ctx.enter_context(
            tc.tile_pool(name="spsum", bufs=3, space="PSUM")
        )

        eps2 = const_pool.tile([P, 1], fp32, tag="eps2")
        nc.vector.memset(eps2[:], EPS * EPS)

        # mask replica [128, B_PER*D] bf16 in SBUF via broadcast DMA
        # (mask is 0/1 so the host-side bf16 cast is exact)
        mbc = const_pool.tile([P, B_PER * D], bf16, tag="mbc")
        for b in range(B_PER):
            nc.sync.dma_start(
                mbc[:, b * D : (b + 1) * D],
                m_d[0:1, b * D : (b + 1) * D].to_broadcast((P, D)),
            )

        def emit_stream(b):
            """DMA loads + mask-mul + square/accum for batch b."""
            n2_b = stat_pool.tile([P, T_TILES], fp32, tag="n2")
            xms = []
            for ti in range(T_TILES):
                xt = x_pool.tile([P, D], fp32)
                r0 = b * S + ti * P
                nc.sync.dma_start(xt[:], x_d[r0 : r0 + P, :])

                xm = xm_pool.tile([P, D], bf16)
                nc.vector.tensor_mul(xm[:], xt[:], mbc[:, b * D : (b + 1) * D])
                xms.append(xm)
                sq = sq_pool.tile([P, D], bf16)
                nc.scalar.activation(
                    sq[:],
                    xm[:],
                    mybir.ActivationFunctionType.Square,
                    accum_out=n2_b[:, ti : ti + 1],
                )
            return n2_b, xms

        def emit_tail(b, n2_b, xms):
            """Norm chain + PE column-sum + output DMAs for batch b.

            Emitted one batch behind emit_stream so the in-order engine
            queues never stall on the cross-engine chain hops.
            """
            nrm = stat_pool.tile([P, T_TILES], fp32, tag="nrm")
            inv_f = stat_pool.tile([P, T_TILES], fp32, tag="invf")
            inv_bf = stat_pool.tile([P, T_TILES], bf16, tag="invbf")
            i2 = stat_pool.tile([P, T_TILES], fp32, tag="i2")
            tr_b = stat_pool.tile([P, T_TILES], fp32, tag="tr")
            # max(n2, eps^2) folded into the sqrt bias: sqrt(n2 + eps^2)
            # equals the clamp at both extremes (n2 >> eps^2 and n2 -> 0);
            # the transition region n2 ~ 1e-16 is unreachable for this data.
            nc.scalar.activation(
                nrm[:], n2_b[:], mybir.ActivationFunctionType.Sqrt,
                bias=eps2[:, :],
            )
            nc.vector.reciprocal(inv_f[:], nrm[:])
            # PE consumes bf16 weights; tr must use the SAME rounded inv so
            # the diagonal inside ||s||^2 cancels exactly.
            nc.vector.tensor_copy(inv_bf[:], inv_f[:])
            nc.vector.tensor_mul(i2[:], inv_bf[:], inv_bf[:])
            nc.vector.tensor_mul(tr_b[:], i2[:], n2_b[:])
            nc.gpsimd.dma_start(
                tr_d[:, b * T_TILES : (b + 1) * T_TILES], tr_b[:]
            )

            # s[d] = sum_t inv_t * xm[t,d] over the 4 row tiles
            sp = spsum_pool.tile([1, D], fp32)
            for ti in range(T_TILES):
                for h in range(2):
                    nc.tensor.matmul(
                        sp[0:1, h * H : (h + 1) * H],
                        inv_bf[:, ti : ti + 1],
                        xms[ti][:, h * H : (h + 1) * H],
                        start=(ti == 0),
                        stop=(ti == T_TILES - 1),
                    )
            s_b = ssb_pool.tile([1, D], fp32, tag="s_sb")
            nc.scalar.copy(s_b[0:1, :], sp[0:1, :])
            nc.gpsimd.dma_start(s_d[0:1, b * D : (b + 1) * D], s_b[0:1, :])

        loop_cm = tc.For_i(0, loop_n, 1) if loop_n > 0 else nullcontext()
        with loop_cm:
            for _rep in range(reps):
                pending = None
                for b in range(B_PER):
                    cur = (b, *emit_stream(b))
                    if pending is not None:
                        emit_tail(*pending)
                    pending = cur
                emit_tail(*pending)

    nc.compile()
    return nc


def _get_nc():
    global _compiled_nc
    if _compiled_nc is None:
        _compiled_nc = _build()
    return _compiled_nc


def _finish(mask_f32: np.ndarray, s_raws: list, trs: list) -> np.ndarray:
    """Host tail: mask s, square-sum, subtract trace, log penalty (f64)."""
    total = 0.0
    for c in range(N_CORES):
        s_raw = np.asarray(s_raws[c], dtype=np.float64).reshape(B_PER, D)
        tr = np.asarray(trs[c], dtype=np.float64)  # [P, N_COLS]
        m = mask_f32[c * B_PER : (c + 1) * B_PER].astype(np.float64)
        sm = s_raw * m
        total += 0.5 * ((sm * sm).sum() - tr.sum())
    count = B * S * (S - 1) // 2
    avg = total / count
    loss = -np.log(1.0 - 0.5 * (avg + 1.0)) * BETA
    return np.asarray(loss, dtype=np.float32)


def kernel(fix_outputs: np.ndarray, region_mask: np.ndarray) -> np.ndarray:
    import ml_dtypes

    from concourse.bass_utils import run_bass_kernel_spmd

    x = np.ascontiguousarray(np.asarray(fix_outputs), dtype=np.float32)
    mask_f32 = np.ascontiguousarray(np.asarray(region_mask).astype(np.float32))
    mask_bf = mask_f32.astype(ml_dtypes.bfloat16)  # 0/1: exact

    nc = _get_nc()
    in_maps = []
    for c in range(N_CORES):
        xs = x[c * B_PER : (c + 1) * B_PER].reshape(B_PER * S, D)
        ms = mask_bf[c * B_PER : (c + 1) * B_PER].reshape(1, B_PER * D)
        in_maps.append({"x": xs, "mask": ms})

    res = run_bass_kernel_spmd(nc, in_maps, list(range(N_CORES)))
    s_raws = [res.results[c]["out_s"] for c in range(N_CORES)]
    trs = [res.results[c]["out_tr"] for c in range(N_CORES)]
    return _finish(mask_f32, s_raws, trs)
